# revision 6
# baseline (speedup 1.0000x reference)
# BitNet GQA attention block on 8 Trainium2 NeuronCores.
#
# Sharding: data parallel over sequence (256 tokens/core). K/V are computed
# per-core for the local tokens, RoPE'd, then AllGathered (one fused fp16
# collective) so every core runs full non-causal attention for its own
# query tokens. Projections are integer-exact mixed-dtype matmuls: 8-bit
# quantized activations as bf16 integers (<=127, exact), ternary weights
# as fp8e4m3 (-1/0/1, exact) to halve the weight DMA stream; PSUM
# accumulates in fp32 and |dot| < 2^24 so results are exact. Attention
# matmuls run in fp16 (full PE rate; f32r measured ~2x slower on HW).
import math

import numpy as np

import concourse.bacc as bacc
import concourse.bass as bass
import concourse.bass_isa as bass_isa
import concourse.mybir as mybir
import concourse.tile as tile
from concourse.masks import make_identity

DT = mybir.dt
AF = mybir.ActivationFunctionType
ALU = mybir.AluOpType
AX = mybir.AxisListType

H = 4096
QH, KVH, D = 32, 8, 128     # query heads, kv heads, head dim
HB = H // 128               # 32 hidden blocks
NREP = QH // KVH
ROUND_C = 12582912.0        # 1.5 * 2**23: fp32 add forces round-to-nearest-even int
LN_EPS = 1e-5
QB = 127.0
SM_SCALE = 1.0 / math.sqrt(128.0)


def build(n_cores=8, s_pc=256, stub_collectives=False, body_reps=1,
          skip_gb=False):
    """Build the SPMD Bass program (identical on all cores; per-core data via inputs)."""
    NT = s_pc // 128            # token tiles per core
    S = s_pc * n_cores
    KT = S // 128               # key-token tiles after gather
    f32, bf16, f32r, f16 = DT.float32, DT.bfloat16, DT.float32r, DT.float16
    f8 = DT.float8e4

    nc = bacc.Bacc("TRN2", target_bir_lowering=False, debug=False, num_devices=n_cores)

    x_d = nc.dram_tensor("x", [s_pc, H], f32, kind="ExternalInput").ap()
    g_d = nc.dram_tensor("lng", [1, H], f32, kind="ExternalInput").ap()
    b_d = nc.dram_tensor("lnb", [1, H], f32, kind="ExternalInput").ap()
    cos_d = nc.dram_tensor("cosT", [D, s_pc], f32, kind="ExternalInput").ap()
    sin_d = nc.dram_tensor("sinTs", [D, s_pc], f32, kind="ExternalInput").ap()
    wq_d = nc.dram_tensor("wqt", [128, QH, HB, 128], f8, kind="ExternalInput").ap()
    wk_d = nc.dram_tensor("wkt", [128, KVH, HB, 128], f8, kind="ExternalInput").ap()
    wv_d = nc.dram_tensor("wvt", [128, HB, KVH * D], f8, kind="ExternalInput").ap()
    wo_d = nc.dram_tensor("wot", [128, HB, H], f8, kind="ExternalInput").ap()
    sc_d = nc.dram_tensor("wscal", [1, 4], f32, kind="ExternalInput").ap()
    onesr_d = nc.dram_tensor("onesr", [128, 1], f32r, kind="ExternalInput").ap()
    y_d = nc.dram_tensor("yT", [s_pc, H], f32, kind="ExternalOutput").ap()

    with tile.TileContext(nc) as tc:
        for rep in range(body_reps):
            _body(nc, tc, n_cores, s_pc, NT, KT,
                  x_d, g_d, b_d, cos_d, sin_d, wq_d, wk_d, wv_d, wo_d, sc_d,
                  onesr_d, y_d, stub_collectives, pfx=f"r{rep}_",
                  skip_gb=skip_gb)
    nc.compile()
    return nc


def _body(nc, tc, n_cores, s_pc, NT, KT,
          x_d, g_d, b_d, cos_d, sin_d, wq_d, wk_d, wv_d, wo_d, sc_d,
          onesr_d, y_d, stub_collectives=False, pfx="", skip_gb=False):
    f32, bf16, f32r, f16 = DT.float32, DT.bfloat16, DT.float32r, DT.float16
    f8 = DT.float8e4
    sync, vec, act, pe, gp = nc.sync, nc.vector, nc.scalar, nc.tensor, nc.gpsimd

    from contextlib import ExitStack

    def bcast_row(psb_pool, ones1, row, out_sb, n, name):
        """Replicate [1, n] row across 128 partitions via K=1 fp32 matmul
        (exact: 1.0 * a) then copy PSUM->SBUF."""
        for i, n0 in enumerate(range(0, n, 512)):
            nn = min(512, n - n0)
            ps = psb_pool.tile([128, 512], f32, name=f"{name}_ps{i}", tag="psb")
            pe.matmul(ps[:, 0:nn], ones1, row[:, n0:n0 + nn],
                      start=True, stop=True)
            vec.tensor_copy(out_sb[:, n0:n0 + nn], ps[:, 0:nn])

    def bcast_from_dram(psb_pool, row_pool, ones1, dram_row, out_sb, n, name):
        """Like bcast_row but streams the source row from DRAM in [1, 512]
        chunks (avoids a [1, n] SBUF tile, which costs n*4 bytes on every
        partition)."""
        for i, n0 in enumerate(range(0, n, 512)):
            nn = min(512, n - n0)
            rt = row_pool.tile([1, 512], f32, name=f"{name}_row{i}", tag="brow")
            sync.dma_start(rt[:, 0:nn], dram_row[:, n0:n0 + nn])
            ps = psb_pool.tile([128, 512], f32, name=f"{name}_ps{i}", tag="psb")
            pe.matmul(ps[:, 0:nn], ones1, rt[:, 0:nn], start=True, stop=True)
            vec.tensor_copy(out_sb[:, n0:n0 + nn], ps[:, 0:nn])

    es = ExitStack()
    with es:
        # ---------------- long-lived pools ----------------
        constp = es.enter_context(tc.tile_pool(name=pfx + "constp", bufs=1))
        dramp = es.enter_context(tc.tile_pool(name=pfx + "dramp", bufs=1, space="DRAM"))
        xTp = es.enter_context(tc.tile_pool(name=pfx + "xTp", bufs=1))
        qTp = es.enter_context(tc.tile_pool(name=pfx + "qTp", bufs=1))
        aop = es.enter_context(tc.tile_pool(name=pfx + "aop", bufs=1))

        cosS = constp.tile([D, s_pc], f32, name="cosS", tag="cosS")
        sinS = constp.tile([D, s_pc], f32, name="sinS", tag="sinS")
        sync.dma_start(cosS, cos_d)
        sync.dma_start(sinS, sin_d)
        ones1 = constp.tile([1, 128], f32, name="ones1", tag="ones1")
        vec.memset(ones1, 1.0)
        ident = constp.tile([128, 128], bf16, name="ident", tag="ident")
        make_identity(nc, ident)
        identf = constp.tile([128, 128], f32, name="identf", tag="identf")
        make_identity(nc, identf)
        scal_sb = constp.tile([128, 4], f32, name="scal_sb", tag="scal_sb")
        scal_row = constp.tile([1, 4], f32, name="scal_row", tag="scal_row")
        sync.dma_start(scal_row, sc_d)
        sw_q, sw_k, sw_v, sw_o = (scal_sb[:, i:i + 1] for i in range(4))
        ones_sb = constp.tile([128, 1], f16, name="ones_sb", tag="ones_sb")
        vec.memset(ones_sb, 1.0)

        # quantized+transposed activations [hid, tok] as bf16 integers
        xT = xTp.tile([128, HB, s_pc], bf16, name="xT", tag="xT")
        # per-token dequant scale r_i = clip(absmax,1e-5)/127, replicated on all partitions
        R = xTp.tile([128, s_pc], f32, name="R", tag="R")
        r_row = constp.tile([1, s_pc], f32, name="r_row", tag="r_row")

        qTall = qTp.tile([128, QH, s_pc], f16, name="qTall", tag="qTall")
        aoall = aop.tile([128, QH, s_pc], f32, name="aoall", tag="aoall")
        acc = aop.tile([128, s_pc], f32, name="acc", tag="acc")
        vec.memset(acc, 0.0)

        # collective buffers: K and V in ONE gather (two sequential gathers
        # serialize on the collective engine with a dead gap between them)
        KVN = KVH * D * s_pc            # == NT * 128 * KVH * D
        kvsrc = dramp.tile([2, KVN], f16, name="kvsrc", tag="kvsrc")
        ksrc = kvsrc[0:1, :].rearrange("o (h d t) -> (o h) d t",
                                       h=KVH, d=D, t=s_pc)
        vsrc = kvsrc[1:2, :].rearrange("o (a p f) -> (o a) p f",
                                       a=NT, p=128, f=KVH * D)
        kv_space = "Local" if stub_collectives else "Shared"
        KVG = dramp.tile([n_cores, 2, KVN], f16, name="KVG", tag="KVG",
                         addr_space=kv_space)
        KG = KVG[:, 0:1, :].rearrange("r o (h d t) -> r (o h) d t",
                                      h=KVH, d=D, t=s_pc)
        VG = KVG[:, 1:2, :].rearrange("r o (a p f) -> r (o a) p f",
                                      a=NT, p=128, f=KVH * D)

        r_tiles = []

        # per-token scale tiles (partition layout) -- live into phase 2
        for t in range(NT):
            r_t = constp.tile([128, 1], f32, name=f"r_{t}", tag=f"r_{t}")
            r_tiles.append(r_t)

        # ---------------- phase 1: layernorm + act quant ----------------
        with tc.tile_pool(name=pfx + "lnp", bufs=2) as lnp, \
             tc.tile_pool(name=pfx + "gbp", bufs=1) as gbp, \
             tc.tile_pool(name=pfx + "statp", bufs=1) as statp, \
             tc.tile_pool(name=pfx + "psb1", bufs=2, space="PSUM") as psb1, \
             tc.tile_pool(name=pfx + "pstp", bufs=4, space="PSUM") as pstp, \
             tc.tile_pool(name=pfx + "xqp", bufs=2) as xqp:
            if not skip_gb:
                Gt = gbp.tile([128, H], f32, name="Gt", tag="Gt")
                Bt = gbp.tile([128, H], f32, name="Bt", tag="Bt")
                bcast_from_dram(psb1, gbp, ones1, g_d, Gt, H, "g")
                bcast_from_dram(psb1, gbp, ones1, b_d, Bt, H, "b")
            bcast_row(psb1, ones1, scal_row, scal_sb, 4, "sc")

            NCH = 4
            CH = H // NCH
            for t in range(NT):
                xs = lnp.tile([128, H], f32, name=f"xs{t}", tag="xs")
                # xq doubles as scratch for the Square activation output
                # (only its accum_out is needed); keeps phase-1 SBUF small
                # so the projection weight pools can prefetch during LN.
                xq = xqp.tile([128, H], bf16, name=f"xq{t}", tag="xq")

                # column-chunked LN so reductions start as DMA chunks land
                # (var computed as E[x^2]-mu^2 instead of E[(x-mu)^2])
                nsum4 = statp.tile([128, NCH], f32, name=f"nsum4{t}",
                                   tag=f"nsum4{t}")
                ss4 = statp.tile([128, NCH], f32, name=f"ss4{t}",
                                 tag=f"ss4{t}")
                for c in range(NCH):
                    sl = slice(c * CH, (c + 1) * CH)
                    sync.dma_start(xs[:, sl],
                                   x_d[t * 128:(t + 1) * 128, sl])
                    vec.tensor_reduce(nsum4[:, c:c + 1], xs[:, sl],
                                      axis=AX.X, op=ALU.add, negate=True)
                    act.activation(xq[:, sl], xs[:, sl], AF.Square,
                                   accum_out=ss4[:, c:c + 1])
                nmu = statp.tile([128, 1], f32, name=f"nmu{t}", tag=f"nmu{t}")
                vec.tensor_reduce(nmu, nsum4, axis=AX.X, op=ALU.add)
                vec.tensor_scalar_mul(nmu, nmu, 1.0 / H)
                sumsq = statp.tile([128, 1], f32, name=f"sumsq{t}", tag=f"sumsq{t}")
                vec.tensor_reduce(sumsq, ss4, axis=AX.X, op=ALU.add)
                mu2 = statp.tile([128, 1], f32, name=f"mu2{t}", tag=f"mu2{t}")
                vec.tensor_mul(mu2, nmu, nmu)
                varv = statp.tile([128, 1], f32, name=f"varv{t}", tag=f"varv{t}")
                vec.tensor_scalar(varv, sumsq, 1.0 / H, LN_EPS, ALU.mult, ALU.add)
                vec.tensor_tensor(varv, varv, mu2, ALU.subtract)
                stdv = statp.tile([128, 1], f32, name=f"stdv{t}", tag=f"stdv{t}")
                act.activation(stdv, varv, AF.Sqrt)
                rstd = statp.tile([128, 1], f32, name=f"rstd{t}", tag=f"rstd{t}")
                vec.reciprocal(rstd, stdv)
                nmr = statp.tile([128, 1], f32, name=f"nmr{t}", tag=f"nmr{t}")
                vec.tensor_mul(nmr, nmu, rstd)
                # normed = x*rstd + (-mu*rstd), then *g + b (in place)
                am4 = statp.tile([128, NCH], f32, name=f"am4{t}", tag=f"am4{t}")
                for c in range(NCH):
                    sl = slice(c * CH, (c + 1) * CH)
                    act.activation(xs[:, sl], xs[:, sl], AF.Identity,
                                   bias=nmr, scale=rstd)
                    if not skip_gb:
                        vec.tensor_mul(xs[:, sl], xs[:, sl], Gt[:, sl])
                        vec.tensor_add(xs[:, sl], xs[:, sl], Bt[:, sl])
                    vec.tensor_reduce(am4[:, c:c + 1], xs[:, sl],
                                      axis=AX.X, op=ALU.max,
                                      apply_absolute_value=True)
                am = statp.tile([128, 1], f32, name=f"am{t}", tag=f"am{t}")
                vec.tensor_reduce(am, am4, axis=AX.X, op=ALU.max)
                amc = statp.tile([128, 1], f32, name=f"amc{t}", tag=f"amc{t}")
                vec.tensor_scalar_max(amc, am, 1e-5)
                r_t = r_tiles[t]
                vec.tensor_scalar_mul(r_t, amc, 1.0 / QB)
                inv = statp.tile([128, 1], f32, name=f"inv{t}", tag=f"inv{t}")
                vec.reciprocal(inv, amc)
                scq = statp.tile([128, 1], f32, name=f"scq{t}", tag=f"scq{t}")
                vec.tensor_scalar_mul(scq, inv, QB)

                # n = round(normed * scq), exact via +C trick; write as bf16 ints
                for c in range(NCH):
                    sl = slice(c * CH, (c + 1) * CH)
                    vec.tensor_scalar(xs[:, sl], xs[:, sl], scq, ROUND_C,
                                      ALU.mult, ALU.add)
                    vec.tensor_scalar_add(xq[:, sl], xs[:, sl], -ROUND_C)

                # transpose into [hid, tok] layout via PE transpose-mode
                # (DMA xbar transpose is ~1.2us per 128x128 and serializes
                # on the Sync queue; PE does it in ~0.3us and stays warm).
                # 4 transposes share one PSUM bank, then one batched copy.
                for h0 in range(0, HB, 4):
                    ps_tp = pstp.tile([128, 4, 128], bf16,
                                      name=f"tp{t}_{h0}", tag="ps_tp")
                    for u in range(4):
                        pe.transpose(ps_tp[:, u, :],
                                     xq[:, (h0 + u) * 128:(h0 + u + 1) * 128],
                                     ident)
                    act.copy(
                        xT[:, h0:h0 + 4, t * 128:(t + 1) * 128],
                        ps_tp)
                # per-token scale row via PE transpose (a DRAM roundtrip
                # here head-of-line blocks the Sync DMA queue behind the
                # LN chain, stalling the whole weight stream)
                ps_r = psb1.tile([1, 128], f32, name=f"ps_r{t}", tag="ps_r")
                pe.transpose(ps_r, r_t, identf)
                vec.tensor_copy(r_row[0:1, t * 128:(t + 1) * 128], ps_r)

            bcast_row(psb1, ones1, r_row, R, s_pc, "r")

        # ---------------- phases 2+3: K,V,Q projections + gathers ----------------
        # One pool scope for all three projections: separate scopes would
        # make the Q-proj weight DMAs wait on SBUF freed by the K/V pools,
        # serializing Q proj behind the V gather.
        with tc.tile_pool(name=pfx + "wkvp", bufs=6) as wkvp, \
             tc.tile_pool(name=pfx + "wvp", bufs=6) as wvp, \
             tc.tile_pool(name=pfx + "wqp", bufs=4) as wqp, \
             tc.tile_pool(name=pfx + "pskv", bufs=2, space="PSUM") as pskv, \
             tc.tile_pool(name=pfx + "psv", bufs=1, space="PSUM") as psvp, \
             tc.tile_pool(name=pfx + "psq", bufs=2, space="PSUM") as psqp, \
             tc.tile_pool(name=pfx + "qdrp", bufs=2) as qdrp, \
             tc.tile_pool(name=pfx + "kdrp", bufs=2) as kdrp:
            # K projection: kT[feat, tok] per kv head
            # (high priority: the gather -- and thus attention start -- is
            # gated on K/V; without this the scheduler interleaves Q-proj
            # matmuls and stretches V proj far past its weight stream)
            hp = tc.high_priority()
            hp.__enter__()
            for f in range(KVH):
                wk_sb = wkvp.tile([128, HB, 128], f8, name=f"wk{f}", tag="wkv")
                sync.dma_start(wk_sb, wk_d[:, f, :, :])
                ps = pskv.tile([128, s_pc], f32, name=f"psk{f}", tag="pskv")
                for k in range(HB):
                    pe.matmul(ps, wk_sb[:, k, :], xT[:, k, :],
                              start=(k == 0), stop=(k == HB - 1))
                kdr = kdrp.tile([128, s_pc], f32, name=f"kdr{f}", tag="kdr")
                vec.scalar_tensor_tensor(kdr, ps, sw_k, R, op0=ALU.mult,
                                         op1=ALU.mult)
                # rope
                rot = kdrp.tile([128, s_pc], f32, name=f"krot{f}", tag="krot")
                gp.dma_start(rot[0:64, :], kdr[64:128, :])
                gp.dma_start(rot[64:128, :], kdr[0:64, :])
                vec.tensor_mul(rot, rot, sinS)
                kcos = kdrp.tile([128, s_pc], f32, name=f"kcos{f}", tag="kcos")
                vec.tensor_mul(kcos, kdr, cosS)
                krp = kdrp.tile([128, s_pc], f16, name=f"krp{f}", tag="krp")
                vec.tensor_add(krp, kcos, rot)
                gp.dma_start(ksrc[f], krp)

            # V projection: v[tok, feat] (x^T as stationary, weights moving)
            psvs = [psvp.tile([128, KVH * D], f32, name=f"psv{t}",
                              tag=f"psv{t}") for t in range(NT)]
            for k in range(HB):
                wv_sb = wvp.tile([128, KVH * D], f8, name=f"wv{k}",
                                 tag="wv")
                act.dma_start(wv_sb, wv_d[:, k, :])
                for t in range(NT):
                    for n0 in (0, 512):
                        pe.matmul(psvs[t][:, n0:n0 + 512],
                                  xT[:, k, t * 128:(t + 1) * 128],
                                  wv_sb[:, n0:n0 + 512],
                                  start=(k == 0), stop=(k == HB - 1))
            for t in range(NT):
                vdr = kdrp.tile([128, KVH * D], f16, name=f"vdr{t}", tag="vdr")
                vec.tensor_scalar(vdr, psvs[t], r_tiles[t], sw_v,
                                  ALU.mult, ALU.mult)
                gp.dma_start(vsrc[t], vdr)

            if stub_collectives:
                for r in range(n_cores):
                    sync.dma_start(KVG[r], kvsrc)
            else:
                gp.collective_compute(
                    "AllGather", ALU.bypass,
                    replica_groups=[list(range(n_cores))],
                    ins=[kvsrc.opt()], outs=[KVG.opt()])
            hp.__exit__(None, None, None)

            # ---- Q projection + rope (overlaps the K/V gather) ----
            for f in range(QH):
                wq_sb = wqp.tile([128, HB, 128], f8, name=f"wq{f}", tag="wq")
                sync.dma_start(wq_sb, wq_d[:, f, :, :])
                ps = psqp.tile([128, s_pc], f32, name=f"psq{f}", tag="psq")
                for k in range(HB):
                    pe.matmul(ps, wq_sb[:, k, :], xT[:, k, :],
                              start=(k == 0), stop=(k == HB - 1))
                qdr = qdrp.tile([128, s_pc], f32, name=f"qdr{f}", tag="qdr")
                vec.scalar_tensor_tensor(qdr, ps, sw_q, R, op0=ALU.mult,
                                         op1=ALU.mult)
                rot = qdrp.tile([128, s_pc], f32, name=f"qrot{f}", tag="qrot")
                gp.dma_start(rot[0:64, :], qdr[64:128, :])
                gp.dma_start(rot[64:128, :], qdr[0:64, :])
                vec.tensor_mul(rot, rot, sinS)
                qcos = qdrp.tile([128, s_pc], f32, name=f"qcos{f}", tag="qcos")
                vec.tensor_mul(qcos, qdr, cosS)
                vec.tensor_add(qTall[:, f, :], qcos, rot)

        # ---------------- phase 4: attention ----------------
        # Query heads processed in PAIRS per kv group: the exp'd scores of
        # two heads live interleaved in one tile [128, KT, 2, s_pc] so the
        # denominator and attention-output matmuls stream N=512 (one MM for
        # both heads) -- halves the PE instruction count and the extra
        # denominator streaming.
        with tc.tile_pool(name=pfx + "kgp", bufs=2) as kgp, \
             tc.tile_pool(name=pfx + "vgp", bufs=2) as vgp, \
             tc.tile_pool(name=pfx + "ep", bufs=4) as ep, \
             tc.tile_pool(name=pfx + "pss", bufs=3, space="PSUM") as pssp, \
             tc.tile_pool(name=pfx + "psd", bufs=1, space="PSUM") as psdp, \
             tc.tile_pool(name=pfx + "pso", bufs=1, space="PSUM") as psop, \
             tc.tile_pool(name=pfx + "dnp", bufs=2) as dnp:
            def attn_tail(g, eT2s, vg_sb):
                """denominators + attention-output + normalize for group g;
                emitted between the NEXT group's score matmuls so the PE
                fills the exp-wait stalls (software pipeline)."""
                psocs = []
                dnrows = []
                for pair in range(2):
                    eT2 = eT2s[pair]
                    psd = psdp.tile([1, 2 * s_pc], f32, name=f"psd{g}_{pair}",
                                    tag="psd")
                    for kt in range(KT):
                        pe.matmul(psd, ones_sb, eT2[:, kt, :, :],
                                  start=(kt == 0), stop=(kt == KT - 1))
                    pso = psop.tile([128, 2 * s_pc], f32,
                                    name=f"pso{g}_{pair}", tag="pso")
                    for kt in range(KT):
                        pe.matmul(pso, vg_sb[:, kt, :], eT2[:, kt, :, :],
                                  start=(kt == 0), stop=(kt == KT - 1))
                    # spill PSUM to SBUF right away so the banks recycle
                    dnrow = dnp.tile([128, 2 * s_pc], f32,
                                      name=f"dnrow{g}_{pair}", tag="dnrow")
                    vec.tensor_copy(dnrow[0:1, :], psd)
                    dnrows.append(dnrow)
                    psoc = dnp.tile([128, 2 * s_pc], f32,
                                    name=f"psoc{g}_{pair}", tag="psoc")
                    vec.tensor_copy(psoc, pso)
                    psocs.append(psoc)
                for pair in range(2):
                    dnrow = dnrows[pair]
                    gp.partition_broadcast(dnrow, dnrow[0:1, :])
                    dnr = dnp.tile([128, 2 * s_pc], f32,
                                   name=f"dnr{g}_{pair}", tag="dnr")
                    vec.reciprocal(dnr, dnrow)
                    for jj in range(2):
                        hq = g * NREP + pair * 2 + jj
                        sl = slice(jj * s_pc, (jj + 1) * s_pc)
                        vec.tensor_tensor(aoall[:, hq, :], psocs[pair][:, sl],
                                          dnr[:, sl], ALU.mult)
                        vec.tensor_tensor(acc, acc, aoall[:, hq, :], ALU.max)
                        vec.scalar_tensor_tensor(acc, aoall[:, hq, :], -1.0,
                                                 acc, op0=ALU.mult,
                                                 op1=ALU.max)

            pending = None
            for g in range(KVH):
                kg_sb = kgp.tile([128, n_cores, s_pc], f16, name=f"kg{g}",
                                 tag="kg")
                gp.dma_start(kg_sb, KG[:, g, :, :].rearrange("r d t -> d r t"))
                vg_sb = vgp.tile([128, KT, 128], f16, name=f"vg{g}", tag="vg")
                for r in range(n_cores):
                    gp.dma_start(
                        vg_sb[:, r * NT:(r + 1) * NT, :],
                        VG[r].rearrange("t p f -> p t f")[:, :,
                                                          g * 128:(g + 1) * 128])
                eT2s = []
                for pair in range(2):
                    eT2 = ep.tile([128, KT, 2, s_pc], f16,
                                  name=f"eT{g}_{pair}", tag="eT")
                    eT2s.append(eT2)
                    qpair = qTall[:, g * NREP + pair * 2:
                                  g * NREP + pair * 2 + 2, :]
                    for kt0 in range(0, KT, 2):
                        pss = pssp.tile([128, 2, 2, s_pc], f32,
                                        name=f"pss{g}_{pair}_{kt0}",
                                        tag="pss")
                        for u in range(2):
                            kt = kt0 + u
                            pe.matmul(pss[:, u, :, :],
                                      kg_sb[:, kt // NT,
                                            (kt % NT) * 128:
                                            (kt % NT) * 128 + 128],
                                      qpair, start=True, stop=True)
                        act.activation(eT2[:, kt0:kt0 + 2, :, :],
                                       pss, AF.Exp, scale=SM_SCALE)
                    if pending is not None and pair == 1:
                        attn_tail(*pending)
                        pending = None
                pending = (g, eT2s, vg_sb)
            attn_tail(*pending)

        # ---------------- phase 5: re-quant + O projection ----------------
        with tc.tile_pool(name=pfx + "q2p", bufs=1) as q2p, \
             tc.tile_pool(name=pfx + "t1p", bufs=2) as t1p, \
             tc.tile_pool(name=pfx + "wop", bufs=4) as wop, \
             tc.tile_pool(name=pfx + "psy", bufs=1, space="PSUM") as psyp, \
             tc.tile_pool(name=pfx + "yp", bufs=2) as yp:
            # per-token absmax over the partition dim in one gpsimd op
            # (acc is already elementwise |.|-accumulated, all >= 0)
            amax2 = q2p.tile([128, s_pc], f32, name="amax2", tag="amax2")
            gp.partition_all_reduce(amax2, acc, 128, bass_isa.ReduceOp.absmax)
            R2 = q2p.tile([128, s_pc], f32, name="R2", tag="R2")
            vec.tensor_scalar(R2, amax2, 1e-5, 1.0 / QB, ALU.max, ALU.mult)
            sc2 = q2p.tile([128, s_pc], f32, name="sc2", tag="sc2")
            vec.reciprocal(sc2, R2)
            # per-token dequant scale on PARTITIONS (output is token-major):
            # scatter R2 row-0 chunks [1,128] -> [128,1] via DMA
            r2t = []
            for t in range(NT):
                r2c = q2p.tile([128, 1], f32, name=f"r2c{t}", tag=f"r2c{t}")
                gp.dma_start(r2c, R2[0:1, t * 128:(t + 1) * 128])
                r2t.append(r2c)
            ao2 = q2p.tile([128, HB, s_pc], bf16, name="ao2", tag="ao2")
            for f in range(QH):
                t1 = t1p.tile([128, s_pc], f32, name=f"t1_{f}", tag="t1")
                vec.tensor_tensor(t1, aoall[:, f, :], sc2, ALU.mult)
                vec.tensor_scalar(ao2[:, f, :], t1, ROUND_C, -ROUND_C,
                                  ALU.add, ALU.add)

            # O projection, x-stationary: y[tok, feat] accumulated over the
            # 32 input blocks; output-feature HALVES so both token tiles'
            # accumulators stay resident (8 PSUM banks) and each ao2
            # LDWEIGHTS feeds 4 N=512 matmuls.
            HH = H // 2
            for half in range(2):
                psys = [psyp.tile([128, HH], f32, name=f"psy{half}_{t}",
                                  tag=f"psy{t}") for t in range(NT)]
                for k4 in range(0, HB, 4):
                    wo_sb = wop.tile([128, 4, HH], f8, name=f"wo{half}_{k4}",
                                     tag="wo")
                    sync.dma_start(wo_sb,
                                   wo_d[:, k4:k4 + 4, half * HH:(half + 1) * HH])
                    for u in range(4):
                        k = k4 + u
                        for t in range(NT):
                            for c in range(0, HH, 512):
                                pe.matmul(psys[t][:, c:c + 512],
                                          ao2[:, k, t * 128:(t + 1) * 128],
                                          wo_sb[:, u, c:c + 512],
                                          start=(k == 0), stop=(k == HB - 1))
                for t in range(NT):
                    y_sb = yp.tile([128, HH], f32, name=f"y{half}_{t}", tag="y")
                    vec.tensor_scalar(y_sb, psys[t], r2t[t], sw_o,
                                      ALU.mult, ALU.mult)
                    gp.dma_start(
                        y_d[t * 128:(t + 1) * 128, half * HH:(half + 1) * HH],
                        y_sb)


# ---------------------------------------------------------------------------
# host side
# ---------------------------------------------------------------------------
def _weight_quant_host(W):
    """Mimic reference _weight_quant: returns ternary m in {-1,0,1} and the
    effective dequant scale (1/scale) as fp32."""
    W = np.asarray(W, dtype=np.float32)
    mean_abs = np.float32(np.mean(np.abs(W), dtype=np.float64))
    clipped = np.maximum(mean_abs, np.float32(1e-5))
    scale = np.float32(1.0) / clipped
    m = np.clip(np.round(W * scale), -1.0, 1.0).astype(np.float32)
    sw = np.float32(1.0) / scale    # dequant scale applied after int matmul
    return m, float(sw)


def _prep_weights(Wq, Wk, Wv, Wo):
    mq, swq = _weight_quant_host(Wq)
    mk, swk = _weight_quant_host(Wk)
    mv, swv = _weight_quant_host(Wv)
    mo, swo = _weight_quant_host(Wo)

    def blocked(mT, fb):  # mT: [H, out] -> [128, fb, HB, 128]
        return np.ascontiguousarray(
            mT.reshape(HB, 128, fb, 128).transpose(1, 2, 0, 3)
        ).astype(np.float32)

    wqt = blocked(mq.T, QH)
    wkt = blocked(mk.T, KVH)
    # O proj is x-stationary: weights are the moving operand, laid out
    # [in_partition, in_block, out] so wot[p, k, o] = Wo[o, k*128+p]
    wot = np.ascontiguousarray(
        mo.T.reshape(HB, 128, H).transpose(1, 0, 2)).astype(np.float32)
    wvt = np.ascontiguousarray(
        mv.T.reshape(HB, 128, KVH * D).transpose(1, 0, 2)).astype(np.float32)
    import ml_dtypes
    tob = lambda a: a.astype(ml_dtypes.float8_e4m3)
    return (tob(wqt), tob(wkt), tob(wvt), tob(wot),
            np.array([[swq, swk, swv, swo]], dtype=np.float32))


def _rope_tables(S):
    inv = (1.0 / (10000.0 ** (np.arange(0, D, 2, dtype=np.float32)
                              / np.float32(D)))).astype(np.float32)
    pos = np.arange(S, dtype=np.float32)
    fr = pos[:, None] * inv[None, :]          # [S, 64]
    emb = np.concatenate([fr, fr], axis=1)    # [S, D]
    cosT = np.cos(emb).T.astype(np.float32).copy()   # [D, S]
    sinT = np.sin(emb).T.astype(np.float32).copy()
    sinT[0:64, :] *= -1.0                      # sign baked for rotate-half
    return cosT, sinT


def _in_maps(inputs, n_cores=8, s_pc=256):
    hs = np.asarray(inputs["hidden_states"], dtype=np.float32)
    ln_w = np.asarray(inputs["ln_w"], dtype=np.float32).reshape(1, H)
    ln_b = np.asarray(inputs["ln_b"], dtype=np.float32).reshape(1, H)
    wqt, wkt, wvt, wot, wscal = _prep_weights(
        inputs["Wq"], inputs["Wk"], inputs["Wv"], inputs["Wo"])
    S = hs.shape[1]
    cosT, sinT = _rope_tables(S)
    maps = []
    for c in range(n_cores):
        sl = slice(c * s_pc, (c + 1) * s_pc)
        maps.append({
            "x": np.ascontiguousarray(hs[0, sl, :]),
            "lng": ln_w, "lnb": ln_b,
            "cosT": np.ascontiguousarray(cosT[:, sl]),
            "sinTs": np.ascontiguousarray(sinT[:, sl]),
            "wqt": wqt, "wkt": wkt, "wvt": wvt, "wot": wot,
            "wscal": wscal,
            "onesr": np.ones((128, 1), dtype=np.float32),
        })
    return maps


_CACHED = {}


def _run(inputs, trace=False, n_cores=8, s_pc=256):
    from concourse.bass_utils import run_bass_kernel_spmd
    skip_gb = bool(
        np.allclose(np.asarray(inputs["ln_w"]), 1.0)
        and np.allclose(np.asarray(inputs["ln_b"]), 0.0))
    key = (n_cores, s_pc, skip_gb)
    if key not in _CACHED:
        _CACHED[key] = build(n_cores, s_pc, skip_gb=skip_gb)
    nc = _CACHED[key]
    maps = _in_maps(inputs, n_cores, s_pc)
    res = run_bass_kernel_spmd(nc, maps, list(range(n_cores)), trace=trace)
    parts = [res.results[c]["yT"] for c in range(n_cores)]
    y = np.concatenate(parts, axis=0)[None, :, :].astype(np.float32)
    return y, res.exec_time_ns


def kernel(**inputs):
    y, _ = _run(inputs, trace=False)
    return y

